# revision 4
# baseline (speedup 1.0000x reference)
"""Trainium2 Bass kernel for char-CNN: 5-tap conv along word_length + max-pool.

Reference computation (per (batch, sentence) word, shapes B=64 S=256 W=20 E=128):
    y[w, e] = sum_{kh=0..4} x[w + kh - 2, e] * conv_w[kh]     (zero padded)
    out[e]  = max_w y[w, e] + conv_b

Strategy:
  - Data-parallel over 8 NeuronCores: 8 batches (2048 words) per core.
  - The conv is a banded 20x20 matrix A applied per word:  Y = A^T-style
    contraction over w_in.  Six words are packed per matmul: stationary
    lhsT = x6 [K=120 (6 words x 20 w_in), M=128 (e)], moving rhs = block
    diagonal A6 [120, 120] -> PSUM [128 (e), 120 (6 words x 20 w_out)].
    fp16 operands (1 cycle/row on PE; fp32 would be 4).
  - Max over w_out is then a free-dim reduce on VectorE straight out of
    PSUM: [128, groups, 20] -> [128, groups] written to a persistent
    [128, 2048] maxima tile; one DMA out at the end (host transposes).
  - ScalarE does the f32 -> f16 cast; engines: DMA in ~21 MB (bound),
    ACT cast, PE conv, DVE max, DMA out 1 MB.
"""

from contextlib import ExitStack

import numpy as np

import concourse.bass as bass
import concourse.mybir as mybir
import concourse.tile as tile
from concourse import bacc

W = 20  # word length
E = 128  # embedding dim
KH = 5  # conv taps
PAD = 2
J = 6  # words per matmul group (6 * 20 = 120 <= 128 partitions)
GP_CHUNK = 16  # groups per DMA chunk (96 words, ~0.98 MiB)
NCORES = 8
BANK = 512  # PSUM bank size in f32 elements


def build_conv_matrix(conv_w: np.ndarray) -> np.ndarray:
    """Block-diagonal [J*W, J*W] matrix: A[j*W+wi, j*W+wo] = conv_w[wi-wo+2]."""
    wv = np.asarray(conv_w, np.float32).reshape(-1)
    assert wv.shape == (KH,)
    blk = np.zeros((W, W), np.float32)
    for wo in range(W):
        for kh in range(KH):
            wi = wo + kh - PAD
            if 0 <= wi < W:
                blk[wi, wo] = wv[kh]
    a = np.zeros((J * W, J * W), np.float32)
    for j in range(J):
        a[j * W : (j + 1) * W, j * W : (j + 1) * W] = blk
    return a.astype(np.float16)


def build_nc(nw: int) -> bass.Bass:
    """Build the per-core Bass graph. nw = words per core."""
    f32 = mybir.dt.float32
    f16 = mybir.dt.float16
    nc = bacc.Bacc()
    x_ext = nc.declare_dram_parameter("x", [nw, W, E], f32, isOutput=False)
    a_ext = nc.declare_dram_parameter("a", [J * W, J * W], f16, isOutput=False)
    out_ext = nc.declare_dram_parameter("out", [E, nw], f32, isOutput=True)

    chunks = []
    w0 = 0
    while w0 < nw:
        n = min(J * GP_CHUNK, nw - w0)
        chunks.append((w0, n))
        w0 += n

    with ExitStack() as ctx:
        tc = ctx.enter_context(tile.TileContext(nc))
        const = ctx.enter_context(tc.tile_pool(name="const", bufs=1))
        xpool = ctx.enter_context(tc.tile_pool(name="xf", bufs=3))
        hpool = ctx.enter_context(tc.tile_pool(name="xh", bufs=3))
        opool = ctx.enter_context(tc.tile_pool(name="o", bufs=1))
        pspool = ctx.enter_context(tc.tile_pool(name="ps", bufs=2, space="PSUM"))

        a_t = const.tile([J * W, J * W], f16)
        nc.sync.dma_start(out=a_t[:, :], in_=a_ext[:, :])
        maxt = opool.tile([E, nw], f32)

        for w0, nwords in chunks:
            ngf = nwords // J  # full groups of J words
            prem = nwords - ngf * J  # leftover words (0..J-1)
            ng = ngf + (1 if prem else 0)
            xt = xpool.tile([J * W, GP_CHUNK * E], f32, tag="xf")
            xh = hpool.tile([J * W, GP_CHUNK * E], f16, tag="xh")
            if ngf:
                src = x_ext[w0 : w0 + ngf * J].rearrange(
                    "(g j) w e -> (j w) g e", j=J
                )
                dst = xt[:, 0 : ngf * E].rearrange("p (g e) -> p g e", e=E)
                nc.sync.dma_start(out=dst, in_=src)
                nc.scalar.copy(xh[:, 0 : ngf * E], xt[:, 0 : ngf * E])
            if prem:
                wp = w0 + ngf * J
                srcp = x_ext[wp : wp + prem].rearrange("j w e -> (j w) e")
                nc.sync.dma_start(
                    out=xt[0 : prem * W, ngf * E : ng * E], in_=srcp
                )
                nc.scalar.copy(
                    xh[0 : prem * W, ngf * E : ng * E],
                    xt[0 : prem * W, ngf * E : ng * E],
                )
            ps = pspool.tile([E, 4 * BANK], f32, tag="ps")
            for g in range(ng):
                p = J if g < ngf else prem
                kp = p * W
                col = (g // 4) * BANK + (g % 4) * (J * W)
                nc.tensor.matmul(
                    ps[:, col : col + kp],
                    lhsT=xh[0:kp, g * E : (g + 1) * E],
                    rhs=a_t[0:kp, 0:kp],
                    start=True,
                    stop=True,
                )
            if prem == 0 and ng % 4 == 0:
                # uniform chunk: one reduce across all banks
                nb = ng // 4
                view = (
                    ps[:, :]
                    .rearrange("p (b x) -> p b x", b=4)[:, 0:nb, 0 : 4 * J * W]
                    .rearrange("p b (c w) -> p b c w", w=W)
                )
                out_v = maxt[:, w0 : w0 + nwords].rearrange(
                    "p (b c) -> p b c", b=nb
                )
                nc.vector.reduce_max(out_v, view, axis=mybir.AxisListType.X)
            else:
                wcur = w0
                for b in range((ng + 3) // 4):
                    glo, ghi = 4 * b, min(4 * b + 4, ng)
                    nbw = sum(J if g < ngf else prem for g in range(glo, ghi))
                    view = ps[:, BANK * b : BANK * b + nbw * W].rearrange(
                        "p (c w) -> p c w", w=W
                    )
                    nc.vector.reduce_max(
                        maxt[:, wcur : wcur + nbw],
                        view,
                        axis=mybir.AxisListType.X,
                    )
                    wcur += nbw
        nc.sync.dma_start(out=out_ext[:, :], in_=maxt[:, :])
    nc.finalize()
    return nc


def kernel(embedded_char, conv_w, conv_b):
    from concourse.bass_utils import run_bass_kernel_spmd

    x = np.asarray(embedded_char, np.float32)
    b_val = float(np.asarray(conv_b, np.float32).reshape(-1)[0])
    B, S, Wl, El = x.shape
    assert (Wl, El) == (W, E)
    bs = B // NCORES
    nw = bs * S
    a16 = build_conv_matrix(conv_w)

    nc = build_nc(nw)
    in_maps = [
        {
            "x": np.ascontiguousarray(x[i * bs : (i + 1) * bs].reshape(nw, W, El)),
            "a": a16,
        }
        for i in range(NCORES)
    ]
    res = run_bass_kernel_spmd(nc, in_maps, core_ids=list(range(NCORES)))
    full = np.concatenate(
        [r["out"].T.reshape(bs, S, El) for r in res.results], axis=0
    )
    if b_val != 0.0:
        full = full + b_val
    return np.ascontiguousarray(full.astype(np.float32))


# revision 5
# speedup vs baseline: 1.0523x; 1.0523x over previous
"""Trainium2 Bass kernel for char-CNN: 5-tap conv along word_length + max-pool.

Reference computation (per (batch, sentence) word, shapes B=64 S=256 W=20 E=128):
    y[w, e] = sum_{kh=0..4} x[w + kh - 2, e] * conv_w[kh]     (zero padded)
    out[e]  = max_w y[w, e] + conv_b

Strategy:
  - Data-parallel over 8 NeuronCores: 8 batches (2048 words) per core.
  - Host pre-arranges each core's shard to z[(j w)=120, group=342, e=128]
    (groups of J=6 words, last group zero-padded) so every DMA descriptor
    is a multi-KiB contiguous run per partition — full HBM bandwidth.
  - The conv is a banded 20x20 matrix applied per word, done on TensorE:
    stationary lhsT = x6 [K=120 (6 words x 20 w_in), M=128 (e)], moving
    rhs = block-diagonal A [120, 120] -> PSUM [128 (e), 120 (6w x 20 w_out)].
    fp16 operands (1 cycle/row on PE; fp32 would be 4).
  - Max over w_out is a free-dim reduce on VectorE straight out of PSUM:
    [128, (groups, 20)] -> [128, groups*6] into a persistent [128, NW]
    maxima tile; one DMA out at the end (host transposes back).
  - ScalarE does the f32 -> f16 cast. Engine budget per core: DMA-in
    ~21 MB (the bound), ACT cast, PE conv, DVE max, DMA-out 1 MB.
"""

from contextlib import ExitStack

import numpy as np

import concourse.bass as bass
import concourse.mybir as mybir
import concourse.tile as tile
from concourse import bacc

W = 20  # word length
E = 128  # embedding dim
KH = 5  # conv taps
PAD = 2
J = 6  # words per matmul group (6 * 20 = 120 <= 128 partitions)
KP = J * W  # contraction size / partitions used (120)
GP_CHUNK = 16  # groups per chunk (96 words, ~0.98 MiB DMA, 4 PSUM banks)
NCORES = 8
BANK = 512  # PSUM bank size in f32 elements


def build_conv_matrix(conv_w: np.ndarray) -> np.ndarray:
    """Block-diagonal [KP, KP] matrix: A[j*W+wi, j*W+wo] = conv_w[wi-wo+2]."""
    wv = np.asarray(conv_w, np.float32).reshape(-1)
    assert wv.shape == (KH,)
    blk = np.zeros((W, W), np.float32)
    for wo in range(W):
        for kh in range(KH):
            wi = wo + kh - PAD
            if 0 <= wi < W:
                blk[wi, wo] = wv[kh]
    a = np.zeros((KP, KP), np.float32)
    for j in range(J):
        a[j * W : (j + 1) * W, j * W : (j + 1) * W] = blk
    return a.astype(np.float16)


def pack_input(x_core: np.ndarray, ng: int) -> np.ndarray:
    """[nw, W, E] f32 -> [KP, ng, E] partition-major, zero-padded to ng*J words."""
    nw = x_core.shape[0]
    xp = np.zeros((ng * J, W, E), np.float32)
    xp[:nw] = x_core
    # (g j) w e -> (j w) g e
    return np.ascontiguousarray(
        xp.reshape(ng, J, W, E).transpose(1, 2, 0, 3).reshape(KP, ng, E)
    )


def build_nc(nw: int) -> bass.Bass:
    """Build the per-core Bass graph. nw = real words per core."""
    f32 = mybir.dt.float32
    f16 = mybir.dt.float16
    ng = (nw + J - 1) // J  # padded group count
    nwp = ng * J  # padded word count

    nc = bacc.Bacc()
    z_ext = nc.declare_dram_parameter("z", [KP, ng, E], f32, isOutput=False)
    a_ext = nc.declare_dram_parameter("a", [KP, KP], f16, isOutput=False)
    out_ext = nc.declare_dram_parameter("out", [E, nw], f32, isOutput=True)

    with ExitStack() as ctx:
        tc = ctx.enter_context(tile.TileContext(nc))
        const = ctx.enter_context(tc.tile_pool(name="const", bufs=1))
        xpool = ctx.enter_context(tc.tile_pool(name="xf", bufs=3))
        hpool = ctx.enter_context(tc.tile_pool(name="xh", bufs=3))
        opool = ctx.enter_context(tc.tile_pool(name="o", bufs=1))
        pspool = ctx.enter_context(tc.tile_pool(name="ps", bufs=2, space="PSUM"))

        a_t = const.tile([KP, KP], f16)
        nc.sync.dma_start(out=a_t[:, :], in_=a_ext[:, :])
        maxt = opool.tile([E, nwp], f32)

        g0 = 0
        while g0 < ng:
            gn = min(GP_CHUNK, ng - g0)  # groups this chunk
            xt = xpool.tile([KP, GP_CHUNK * E], f32, tag="xf")
            xh = hpool.tile([KP, GP_CHUNK * E], f16, tag="xh")
            src = z_ext[:, g0 : g0 + gn, :].rearrange("p g e -> p (g e)")
            nc.sync.dma_start(out=xt[:, 0 : gn * E], in_=src)
            nc.scalar.copy(xh[:, 0 : gn * E], xt[:, 0 : gn * E])
            ps = pspool.tile([E, 4 * BANK], f32, tag="ps")
            for g in range(gn):
                col = (g // 4) * BANK + (g % 4) * KP
                nc.tensor.matmul(
                    ps[:, col : col + KP],
                    lhsT=xh[:, g * E : (g + 1) * E],
                    rhs=a_t[:, :],
                    start=True,
                    stop=True,
                )
            nbank = (gn + 3) // 4
            if gn % 4 == 0:
                view = (
                    ps[:, :]
                    .rearrange("p (b x) -> p b x", b=4)[:, 0:nbank, 0 : 4 * KP]
                    .rearrange("p b (c w) -> p b c w", w=W)
                )
                out_v = maxt[:, g0 * J : (g0 + gn) * J].rearrange(
                    "p (b c) -> p b c", b=nbank
                )
                nc.vector.reduce_max(out_v, view, axis=mybir.AxisListType.X)
            else:
                wcur = g0 * J
                for b in range(nbank):
                    gb = min(4, gn - 4 * b)  # groups in this bank
                    view = ps[:, BANK * b : BANK * b + gb * KP].rearrange(
                        "p (c w) -> p c w", w=W
                    )
                    nc.vector.reduce_max(
                        maxt[:, wcur : wcur + gb * J],
                        view,
                        axis=mybir.AxisListType.X,
                    )
                    wcur += gb * J
            g0 += gn
        nc.sync.dma_start(out=out_ext[:, :], in_=maxt[:, 0:nw])
    nc.finalize()
    return nc


def kernel(embedded_char, conv_w, conv_b):
    from concourse.bass_utils import run_bass_kernel_spmd

    x = np.asarray(embedded_char, np.float32)
    b_val = float(np.asarray(conv_b, np.float32).reshape(-1)[0])
    B, S, Wl, El = x.shape
    assert (Wl, El) == (W, E)
    bs = B // NCORES
    nw = bs * S
    ng = (nw + J - 1) // J
    a16 = build_conv_matrix(conv_w)

    nc = build_nc(nw)
    in_maps = [
        {
            "z": pack_input(x[i * bs : (i + 1) * bs].reshape(nw, Wl, El), ng),
            "a": a16,
        }
        for i in range(NCORES)
    ]
    res = run_bass_kernel_spmd(nc, in_maps, core_ids=list(range(NCORES)))
    full = np.concatenate(
        [r["out"].T.reshape(bs, S, El) for r in res.results], axis=0
    )
    if b_val != 0.0:
        full = full + b_val
    return np.ascontiguousarray(full.astype(np.float32))


# revision 8
# speedup vs baseline: 1.6691x; 1.5862x over previous
"""Trainium2 Bass kernel for char-CNN: 5-tap conv along word_length + max-pool.

Reference computation (per (batch, sentence) word, shapes B=64 S=256 W=20 E=128):
    y[w, e] = sum_{kh=0..4} x[w + kh - 2, e] * conv_w[kh]     (zero padded)
    out[e]  = max_w y[w, e] + conv_b

Strategy:
  - Data-parallel over 8 NeuronCores: 8 batches (2048 words) per core.
  - Host pre-arranges each core's shard to z[(j w)=120, group=342, e=128]
    (groups of J=6 words, last group zero-padded) so every DMA descriptor
    is a multi-KiB contiguous run per partition — full HBM bandwidth.
  - The conv is a banded 20x20 matrix applied per word, done on TensorE:
    stationary lhsT = x6 [K=120 (6 words x 20 w_in), M=128 (e)], moving
    rhs = block-diagonal A [120, 120] -> PSUM [128 (e), 120 (6w x 20 w_out)].
    fp16 operands (1 cycle/row on PE; fp32 would be 4).
  - Max over w_out is a free-dim reduce on VectorE straight out of PSUM:
    [128, (groups, 20)] -> [128, groups*6] into a persistent [128, NW]
    maxima tile; one DMA out at the end (host transposes back).
  - Input DMAs are spread across the SP-HWDGE / ACT-HWDGE / SWDGE rings so
    the 16 SDMA engines always have in-flight work (one FIFO ring alone
    leaves completion-latency bubbles).  The SWDGE (gpsimd) ring casts
    f32 -> f16 in the DMA datapath; HWDGE rings land f32 and ScalarE casts.
"""

from contextlib import ExitStack

import numpy as np

import concourse.bass as bass
import concourse.mybir as mybir
import concourse.tile as tile
from concourse import bacc

W = 20  # word length
E = 128  # embedding dim
KH = 5  # conv taps
PAD = 2
J = 6  # words per matmul group (6 * 20 = 120 <= 128 partitions)
KP = J * W  # contraction size / partitions used (120)
CG = 16  # groups per compute sub-chunk (4 PSUM banks)
NCORES = 8
BANK = 512  # PSUM bank size in f32 elements


def build_conv_matrix(conv_w: np.ndarray) -> np.ndarray:
    """Block-diagonal [KP, KP] matrix: A[j*W+wi, j*W+wo] = conv_w[wi-wo+2]."""
    wv = np.asarray(conv_w, np.float32).reshape(-1)
    assert wv.shape == (KH,)
    blk = np.zeros((W, W), np.float32)
    for wo in range(W):
        for kh in range(KH):
            wi = wo + kh - PAD
            if 0 <= wi < W:
                blk[wi, wo] = wv[kh]
    a = np.zeros((KP, KP), np.float32)
    for j in range(J):
        a[j * W : (j + 1) * W, j * W : (j + 1) * W] = blk
    return a.astype(np.float16)


def pack_input(x_core: np.ndarray, ng: int) -> np.ndarray:
    """[nw, W, E] f32 -> [KP, ng, E] partition-major, zero-padded to ng*J words."""
    nw = x_core.shape[0]
    xp = np.zeros((ng * J, W, E), np.float32)
    xp[:nw] = x_core
    # (g j) w e -> (j w) g e
    return np.ascontiguousarray(
        xp.reshape(ng, J, W, E).transpose(1, 2, 0, 3).reshape(KP, ng, E)
    )


def chunk_plan(ng: int) -> list[int]:
    """Descending chunk sizes: big early (fewer ring bubbles while the
    stream is deep), small at the end (short pipeline tail)."""
    sizes = []
    rem = ng
    for sz, keep in ((64, 96), (32, 48), (16, 24), (8, 8)):
        while rem >= max(sz, keep):
            sizes.append(sz)
            rem -= sz
    if rem:
        sizes.append(rem)
    return sizes


def build_nc(
    nw: int,
    dma_rings: tuple[str, ...] = ("gpsimd",),
    bufs: int = 5,
) -> bass.Bass:
    """Build the per-core Bass graph. nw = real words per core.

    dma_rings: which descriptor rings carry the input stream, round-robin
    per chunk. 'gpsimd' (SWDGE) casts f32->f16 during the DMA; HWDGE rings
    ('sync'/'scalar') land f32 and ScalarE casts to f16.
    """
    f32 = mybir.dt.float32
    f16 = mybir.dt.float16
    ng = (nw + J - 1) // J  # padded group count
    nwp = ng * J  # padded word count

    nc = bacc.Bacc()
    z_ext = nc.declare_dram_parameter("z", [KP, ng, E], f32, isOutput=False)
    a_ext = nc.declare_dram_parameter("a", [KP, KP], f16, isOutput=False)
    out_ext = nc.declare_dram_parameter("out", [E, nw], f32, isOutput=True)

    engines = {
        "sync": nc.sync,
        "scalar": nc.scalar,
        "gpsimd": nc.gpsimd,
    }

    with ExitStack() as ctx:
        tc = ctx.enter_context(tile.TileContext(nc))
        const = ctx.enter_context(tc.tile_pool(name="const", bufs=1))
        xpool = ctx.enter_context(tc.tile_pool(name="xf", bufs=3))
        hpool = ctx.enter_context(tc.tile_pool(name="xh", bufs=bufs))
        opool = ctx.enter_context(tc.tile_pool(name="o", bufs=1))
        pspool = ctx.enter_context(tc.tile_pool(name="ps", bufs=2, space="PSUM"))

        a_t = const.tile([KP, KP], f16)
        nc.sync.dma_start(out=a_t[:, :], in_=a_ext[:, :])
        maxt = opool.tile([E, nwp], f32)

        def compute(xh, coff, g0, gn):
            """Matmuls + max-reduce for gn (<=CG) groups; xh f16 tile,
            coff = column offset (elements) of group g0 inside xh."""
            ps = pspool.tile([E, 4 * BANK], f32, tag="ps")
            for g in range(gn):
                col = (g // 4) * BANK + (g % 4) * KP
                nc.tensor.matmul(
                    ps[:, col : col + KP],
                    lhsT=xh[:, coff + g * E : coff + (g + 1) * E],
                    rhs=a_t[:, :],
                    start=True,
                    stop=True,
                )
            nbank = (gn + 3) // 4
            if gn % 4 == 0:
                view = (
                    ps[:, :]
                    .rearrange("p (b x) -> p b x", b=4)[:, 0:nbank, 0 : 4 * KP]
                    .rearrange("p b (c w) -> p b c w", w=W)
                )
                out_v = maxt[:, g0 * J : (g0 + gn) * J].rearrange(
                    "p (b c) -> p b c", b=nbank
                )
                nc.vector.reduce_max(out_v, view, axis=mybir.AxisListType.X)
            else:
                wcur = g0 * J
                for b in range(nbank):
                    gb = min(4, gn - 4 * b)
                    view = ps[:, BANK * b : BANK * b + gb * KP].rearrange(
                        "p (c w) -> p c w", w=W
                    )
                    nc.vector.reduce_max(
                        maxt[:, wcur : wcur + gb * J],
                        view,
                        axis=mybir.AxisListType.X,
                    )
                    wcur += gb * J

        g0 = 0
        sizes = chunk_plan(ng)
        max_gn = max(sizes)
        for ring, gn in enumerate(sizes):
            eng_name = dma_rings[ring % len(dma_rings)]
            src = z_ext[:, g0 : g0 + gn, :].rearrange("p g e -> p (g e)")
            xh = hpool.tile([KP, max_gn * E], f16, tag="xh")
            if eng_name == "gpsimd":
                # SWDGE casts f32 -> f16 inside the DMA datapath
                engines[eng_name].dma_start(out=xh[:, 0 : gn * E], in_=src)
            else:
                xt = xpool.tile([KP, max_gn * E], f32, tag="xf")
                engines[eng_name].dma_start(out=xt[:, 0 : gn * E], in_=src)
                nc.scalar.copy(xh[:, 0 : gn * E], xt[:, 0 : gn * E])
            for s0 in range(0, gn, CG):
                sn = min(CG, gn - s0)
                compute(xh, s0 * E, g0 + s0, sn)
            # stream this chunk's slice of the output while the input
            # stream continues (sync ring is otherwise idle)
            w_lo = g0 * J
            w_hi = min((g0 + gn) * J, nw)
            if w_lo < nw:
                nc.sync.dma_start(
                    out=out_ext[:, w_lo:w_hi], in_=maxt[:, w_lo:w_hi]
                )
            g0 += gn
    nc.finalize()
    return nc


def kernel(embedded_char, conv_w, conv_b):
    from concourse.bass_utils import run_bass_kernel_spmd

    x = np.asarray(embedded_char, np.float32)
    b_val = float(np.asarray(conv_b, np.float32).reshape(-1)[0])
    B, S, Wl, El = x.shape
    assert (Wl, El) == (W, E)
    bs = B // NCORES
    nw = bs * S
    ng = (nw + J - 1) // J
    a16 = build_conv_matrix(conv_w)

    nc = build_nc(nw)
    in_maps = [
        {
            "z": pack_input(x[i * bs : (i + 1) * bs].reshape(nw, Wl, El), ng),
            "a": a16,
        }
        for i in range(NCORES)
    ]
    res = run_bass_kernel_spmd(nc, in_maps, core_ids=list(range(NCORES)))
    full = np.concatenate(
        [r["out"].T.reshape(bs, S, El) for r in res.results], axis=0
    )
    if b_val != 0.0:
        full = full + b_val
    return np.ascontiguousarray(full.astype(np.float32))


# revision 18
# speedup vs baseline: 1.7604x; 1.0547x over previous
"""Trainium2 Bass kernel for char-CNN: 5-tap conv along word_length + max-pool.

Reference computation (per (batch, sentence) word, shapes B=64 S=256 W=20 E=128):
    y[w, e] = sum_{kh=0..4} x[w + kh - 2, e] * conv_w[kh]     (zero padded)
    out[e]  = max_w y[w, e] + conv_b

Strategy:
  - Data-parallel over 8 NeuronCores: 8 batches (2048 words) per core.
  - Host pre-arranges each core's shard to z[(j w)=120, group=342, e=128]
    (groups of J=6 words, last group zero-padded) so every DMA descriptor
    is a multi-KiB contiguous run per partition — full HBM bandwidth.
  - The conv is a banded 20x20 matrix applied per word, done on TensorE:
    stationary lhsT = x6 [K=120 (6 words x 20 w_in), M=128 (e)], moving
    rhs = block-diagonal A [120, 120] -> PSUM [128 (e), 120 (6w x 20 w_out)].
    fp16 operands (1 cycle/row on PE; fp32 would be 4).
  - Max over w_out is a free-dim reduce on VectorE straight out of PSUM:
    [128, (groups, 20)] -> [128, groups*6] into a persistent [128, NW]
    maxima tile; one DMA out at the end (host transposes back).
  - Input DMAs are spread across the SP-HWDGE / ACT-HWDGE / SWDGE rings so
    the 16 SDMA engines always have in-flight work (one FIFO ring alone
    leaves completion-latency bubbles).  The SWDGE (gpsimd) ring casts
    f32 -> f16 in the DMA datapath; HWDGE rings land f32 and ScalarE casts.
"""

from contextlib import ExitStack

import numpy as np

import concourse.bass as bass
import concourse.mybir as mybir
import concourse.tile as tile
from concourse import bacc

W = 20  # word length
E = 128  # embedding dim
KH = 5  # conv taps
PAD = 2
J = 6  # words per matmul group (6 * 20 = 120 <= 128 partitions)
KP = J * W  # contraction size / partitions used (120)
CG = 16  # groups per compute sub-chunk (4 PSUM banks)
NCORES = 8
BANK = 512  # PSUM bank size in f32 elements


def build_conv_matrix(conv_w: np.ndarray) -> np.ndarray:
    """Block-diagonal [KP, KP] matrix: A[j*W+wi, j*W+wo] = conv_w[wi-wo+2]."""
    wv = np.asarray(conv_w, np.float32).reshape(-1)
    assert wv.shape == (KH,)
    blk = np.zeros((W, W), np.float32)
    for wo in range(W):
        for kh in range(KH):
            wi = wo + kh - PAD
            if 0 <= wi < W:
                blk[wi, wo] = wv[kh]
    a = np.zeros((KP, KP), np.float32)
    for j in range(J):
        a[j * W : (j + 1) * W, j * W : (j + 1) * W] = blk
    return a.astype(np.float16)


def pack_input(x_core: np.ndarray, ng: int) -> np.ndarray:
    """[nw, W, E] f32 -> [KP, ng, E] partition-major, zero-padded to ng*J words."""
    nw = x_core.shape[0]
    xp = np.zeros((ng * J, W, E), np.float32)
    xp[:nw] = x_core
    # (g j) w e -> (j w) g e
    return np.ascontiguousarray(
        xp.reshape(ng, J, W, E).transpose(1, 2, 0, 3).reshape(KP, ng, E)
    )


def chunk_plan(ng: int, big: int = 64) -> list[int]:
    """Descending chunk sizes: big early (fewer ring bubbles while the
    stream is deep), small at the end (short pipeline tail)."""
    sizes = []
    rem = ng
    for sz, keep in ((64, 96), (32, 48), (16, 24), (8, 8)):
        if sz > big:
            continue
        while rem >= max(sz, keep):
            sizes.append(sz)
            rem -= sz
    if rem:
        sizes.append(rem)
    return sizes


def build_nc(
    nw: int,
    dma_rings: tuple[str, ...] = ("gpsimd",),
    bufs: int = 22,
    first_ring: str | None = None,
    big_chunk: int = 16,
    cg: int = 8,
    psum_bufs: int = 4,
) -> bass.Bass:
    """Build the per-core Bass graph. nw = real words per core.

    dma_rings: which descriptor rings carry the input stream, round-robin
    per chunk. 'gpsimd' (SWDGE) casts f32->f16 during the DMA; HWDGE rings
    ('sync'/'scalar') land f32 and ScalarE casts to f16.
    """
    f32 = mybir.dt.float32
    f16 = mybir.dt.float16
    ng = (nw + J - 1) // J  # padded group count
    nwp = ng * J  # padded word count

    nc = bacc.Bacc()
    z_ext = nc.declare_dram_parameter("z", [KP, ng, E], f32, isOutput=False)
    a_ext = nc.declare_dram_parameter("a", [KP, KP], f16, isOutput=False)
    out_ext = nc.declare_dram_parameter("out", [E, nw], f32, isOutput=True)

    engines = {
        "sync": nc.sync,
        "scalar": nc.scalar,
        "gpsimd": nc.gpsimd,
    }

    with ExitStack() as ctx:
        tc = ctx.enter_context(tile.TileContext(nc))
        const = ctx.enter_context(tc.tile_pool(name="const", bufs=1))
        xpool = ctx.enter_context(tc.tile_pool(name="xf", bufs=3))
        hpool = ctx.enter_context(tc.tile_pool(name="xh", bufs=bufs))
        opool = ctx.enter_context(tc.tile_pool(name="o", bufs=1))
        pspool = ctx.enter_context(
            tc.tile_pool(name="ps", bufs=psum_bufs, space="PSUM")
        )
        ps_banks = (cg + 3) // 4  # PSUM banks per compute sub-chunk

        a_t = const.tile([KP, KP], f16)
        nc.sync.dma_start(out=a_t[:, :], in_=a_ext[:, :])
        maxt = opool.tile([E, nwp], f32)

        def compute(xh, coff, g0, gn):
            """Matmuls + max-reduce for gn (<=cg) groups; xh f16 tile,
            coff = column offset (elements) of group g0 inside xh."""
            ps = pspool.tile([E, ps_banks * BANK], f32, tag="ps")
            for g in range(gn):
                col = (g // 4) * BANK + (g % 4) * KP
                nc.tensor.matmul(
                    ps[:, col : col + KP],
                    lhsT=xh[:, coff + g * E : coff + (g + 1) * E],
                    rhs=a_t[:, :],
                    start=True,
                    stop=True,
                )
            nbank = (gn + 3) // 4
            if gn % 4 == 0:
                view = (
                    ps[:, 0 : nbank * BANK]
                    .rearrange("p (b x) -> p b x", b=nbank)[:, :, 0 : 4 * KP]
                    .rearrange("p b (c w) -> p b c w", w=W)
                )
                out_v = maxt[:, g0 * J : (g0 + gn) * J].rearrange(
                    "p (b c) -> p b c", b=nbank
                )
                nc.vector.reduce_max(out_v, view, axis=mybir.AxisListType.X)
            else:
                wcur = g0 * J
                for b in range(nbank):
                    gb = min(4, gn - 4 * b)
                    view = ps[:, BANK * b : BANK * b + gb * KP].rearrange(
                        "p (c w) -> p c w", w=W
                    )
                    nc.vector.reduce_max(
                        maxt[:, wcur : wcur + gb * J],
                        view,
                        axis=mybir.AxisListType.X,
                    )
                    wcur += gb * J

        g0 = 0
        if first_ring is not None:
            sizes = [16] + chunk_plan(ng - 16, big_chunk)
            rings = [first_ring] + [
                dma_rings[i % len(dma_rings)] for i in range(len(sizes) - 1)
            ]
        else:
            sizes = chunk_plan(ng, big_chunk)
            rings = [dma_rings[i % len(dma_rings)] for i in range(len(sizes))]
        max_gn = max(sizes)
        for ring, gn in enumerate(sizes):
            eng_name = rings[ring]
            src = z_ext[:, g0 : g0 + gn, :].rearrange("p g e -> p (g e)")
            xh = hpool.tile([KP, max_gn * E], f16, tag="xh")
            if eng_name == "gpsimd":
                # SWDGE casts f32 -> f16 inside the DMA datapath
                engines[eng_name].dma_start(out=xh[:, 0 : gn * E], in_=src)
            else:
                xt = xpool.tile([KP, max_gn * E], f32, tag="xf")
                engines[eng_name].dma_start(out=xt[:, 0 : gn * E], in_=src)
                nc.scalar.copy(xh[:, 0 : gn * E], xt[:, 0 : gn * E])
            for s0 in range(0, gn, cg):
                sn = min(cg, gn - s0)
                compute(xh, s0 * E, g0 + s0, sn)
            # stream this chunk's slice of the output while the input
            # stream continues (sync ring is otherwise idle)
            w_lo = g0 * J
            w_hi = min((g0 + gn) * J, nw)
            if w_lo < nw:
                nc.sync.dma_start(
                    out=out_ext[:, w_lo:w_hi], in_=maxt[:, w_lo:w_hi]
                )
            g0 += gn
    nc.finalize()
    return nc


def kernel(embedded_char, conv_w, conv_b):
    from concourse.bass_utils import run_bass_kernel_spmd

    x = np.asarray(embedded_char, np.float32)
    b_val = float(np.asarray(conv_b, np.float32).reshape(-1)[0])
    B, S, Wl, El = x.shape
    assert (Wl, El) == (W, E)
    bs = B // NCORES
    nw = bs * S
    ng = (nw + J - 1) // J
    a16 = build_conv_matrix(conv_w)

    nc = build_nc(nw)
    in_maps = [
        {
            "z": pack_input(x[i * bs : (i + 1) * bs].reshape(nw, Wl, El), ng),
            "a": a16,
        }
        for i in range(NCORES)
    ]
    res = run_bass_kernel_spmd(nc, in_maps, core_ids=list(range(NCORES)))
    full = np.concatenate(
        [r["out"].T.reshape(bs, S, El) for r in res.results], axis=0
    )
    if b_val != 0.0:
        full = full + b_val
    return np.ascontiguousarray(full.astype(np.float32))


# revision 23
# speedup vs baseline: 1.9837x; 1.1268x over previous
"""Trainium2 Bass kernel for char-CNN: 5-tap conv along word_length + max-pool.

Reference computation (per (batch, sentence) word, shapes B=64 S=256 W=20 E=128):
    y[w, e] = sum_{kh=0..4} x[w + kh - 2, e] * conv_w[kh]     (zero padded)
    out[e]  = max_w y[w, e] + conv_b

Strategy:
  - Data-parallel over 8 NeuronCores: 8 batches (2048 words) per core.
  - Host pre-arranges each core's shard to z[(j w)=120, group=342, e=128]
    (groups of J=6 words, last group zero-padded) so every DMA descriptor
    is a multi-KiB contiguous run per partition — full HBM bandwidth.
  - The conv is a banded 20x20 matrix applied per word, done on TensorE:
    stationary lhsT = x6 [K=120 (6 words x 20 w_in), M=128 (e)], moving
    rhs = block-diagonal A [120, 120] -> PSUM [128 (e), 120 (6w x 20 w_out)].
    fp16 operands (1 cycle/row on PE; fp32 would be 4).
  - Max over w_out is a free-dim reduce on VectorE straight out of PSUM:
    [128, (groups, 20)] -> [128, groups*6] into a persistent [128, NW]
    maxima tile; one DMA out at the end (host transposes back).
  - Input DMAs are spread across the SP-HWDGE / ACT-HWDGE / SWDGE rings so
    the 16 SDMA engines always have in-flight work (one FIFO ring alone
    leaves completion-latency bubbles).  The SWDGE (gpsimd) ring casts
    f32 -> f16 in the DMA datapath; HWDGE rings land f32 and ScalarE casts.
"""

from contextlib import ExitStack

import numpy as np

import concourse.bass as bass
import concourse.mybir as mybir
import concourse.tile as tile
from concourse import bacc

W = 20  # word length
E = 128  # embedding dim
KH = 5  # conv taps
PAD = 2
J = 6  # words per matmul group (6 * 20 = 120 <= 128 partitions)
KP = J * W  # contraction size / partitions used (120)
CG = 16  # groups per compute sub-chunk (4 PSUM banks)
NCORES = 8
BANK = 512  # PSUM bank size in f32 elements


def build_conv_matrix(conv_w: np.ndarray) -> np.ndarray:
    """Block-diagonal [KP, KP] matrix: A[j*W+wi, j*W+wo] = conv_w[wi-wo+2]."""
    wv = np.asarray(conv_w, np.float32).reshape(-1)
    assert wv.shape == (KH,)
    blk = np.zeros((W, W), np.float32)
    for wo in range(W):
        for kh in range(KH):
            wi = wo + kh - PAD
            if 0 <= wi < W:
                blk[wi, wo] = wv[kh]
    a = np.zeros((KP, KP), np.float32)
    for j in range(J):
        a[j * W : (j + 1) * W, j * W : (j + 1) * W] = blk
    return a.astype(np.float16)


def pack_input(x_core: np.ndarray, ng: int) -> np.ndarray:
    """[nw, W, E] f32 -> [KP, ng, E] f16 partition-major, zero-padded to
    ng*J words. The fp16 cast is the same one the kernel's compute path
    uses (TensorE consumes fp16); doing it host-side halves HBM traffic."""
    nw = x_core.shape[0]
    xp = np.zeros((ng * J, W, E), np.float16)
    xp[:nw] = x_core.astype(np.float16)
    # (g j) w e -> (j w) g e
    return np.ascontiguousarray(
        xp.reshape(ng, J, W, E).transpose(1, 2, 0, 3).reshape(KP, ng, E)
    )


def chunk_plan(ng: int, big: int = 64) -> list[int]:
    """Descending chunk sizes: big early (fewer ring bubbles while the
    stream is deep), small at the end (short pipeline tail)."""
    sizes = []
    rem = ng
    for sz, keep in ((64, 96), (32, 48), (16, 24), (8, 8)):
        if sz > big:
            continue
        while rem >= max(sz, keep):
            sizes.append(sz)
            rem -= sz
    if rem:
        sizes.append(rem)
    return sizes


def build_nc(
    nw: int,
    dma_rings: tuple[str, ...] = ("gpsimd",),
    bufs: int = 22,
    first_ring: str | None = None,
    big_chunk: int = 16,
    cg: int = 8,
    psum_bufs: int = 4,
) -> bass.Bass:
    """Build the per-core Bass graph. nw = real words per core.

    dma_rings: which descriptor rings carry the input stream, round-robin
    per chunk. 'gpsimd' (SWDGE) casts f32->f16 during the DMA; HWDGE rings
    ('sync'/'scalar') land f32 and ScalarE casts to f16.
    """
    f32 = mybir.dt.float32
    f16 = mybir.dt.float16
    ng = (nw + J - 1) // J  # padded group count
    nwp = ng * J  # padded word count

    nc = bacc.Bacc()
    z_ext = nc.declare_dram_parameter("z", [KP, ng, E], f16, isOutput=False)
    a_ext = nc.declare_dram_parameter("a", [KP, KP], f16, isOutput=False)
    out_ext = nc.declare_dram_parameter("out", [E, nw], f32, isOutput=True)

    engines = {
        "sync": nc.sync,
        "scalar": nc.scalar,
        "gpsimd": nc.gpsimd,
    }

    with ExitStack() as ctx:
        tc = ctx.enter_context(tile.TileContext(nc))
        const = ctx.enter_context(tc.tile_pool(name="const", bufs=1))
        hpool = ctx.enter_context(tc.tile_pool(name="xh", bufs=bufs))
        opool = ctx.enter_context(tc.tile_pool(name="o", bufs=1))
        pspool = ctx.enter_context(
            tc.tile_pool(name="ps", bufs=psum_bufs, space="PSUM")
        )
        ps_banks = (cg + 3) // 4  # PSUM banks per compute sub-chunk

        a_t = const.tile([KP, KP], f16)
        nc.sync.dma_start(out=a_t[:, :], in_=a_ext[:, :])
        maxt = opool.tile([E, nwp], f32)

        def compute(xh, coff, g0, gn):
            """Matmuls + max-reduce for gn (<=cg) groups; xh f16 tile,
            coff = column offset (elements) of group g0 inside xh."""
            ps = pspool.tile([E, ps_banks * BANK], f32, tag="ps")
            for g in range(gn):
                col = (g // 4) * BANK + (g % 4) * KP
                nc.tensor.matmul(
                    ps[:, col : col + KP],
                    lhsT=xh[:, coff + g * E : coff + (g + 1) * E],
                    rhs=a_t[:, :],
                    start=True,
                    stop=True,
                )
            nbank = (gn + 3) // 4
            if gn % 4 == 0:
                view = (
                    ps[:, 0 : nbank * BANK]
                    .rearrange("p (b x) -> p b x", b=nbank)[:, :, 0 : 4 * KP]
                    .rearrange("p b (c w) -> p b c w", w=W)
                )
                out_v = maxt[:, g0 * J : (g0 + gn) * J].rearrange(
                    "p (b c) -> p b c", b=nbank
                )
                nc.vector.reduce_max(out_v, view, axis=mybir.AxisListType.X)
            else:
                wcur = g0 * J
                for b in range(nbank):
                    gb = min(4, gn - 4 * b)
                    view = ps[:, BANK * b : BANK * b + gb * KP].rearrange(
                        "p (c w) -> p c w", w=W
                    )
                    nc.vector.reduce_max(
                        maxt[:, wcur : wcur + gb * J],
                        view,
                        axis=mybir.AxisListType.X,
                    )
                    wcur += gb * J

        g0 = 0
        if first_ring is not None:
            sizes = [16] + chunk_plan(ng - 16, big_chunk)
            rings = [first_ring] + [
                dma_rings[i % len(dma_rings)] for i in range(len(sizes) - 1)
            ]
        else:
            sizes = chunk_plan(ng, big_chunk)
            rings = [dma_rings[i % len(dma_rings)] for i in range(len(sizes))]
        max_gn = max(sizes)
        for ring, gn in enumerate(sizes):
            eng_name = rings[ring]
            src = z_ext[:, g0 : g0 + gn, :].rearrange("p g e -> p (g e)")
            xh = hpool.tile([KP, max_gn * E], f16, tag="xh")
            engines[eng_name].dma_start(out=xh[:, 0 : gn * E], in_=src)
            for s0 in range(0, gn, cg):
                sn = min(cg, gn - s0)
                compute(xh, s0 * E, g0 + s0, sn)
            # stream this chunk's slice of the output while the input
            # stream continues (sync ring is otherwise idle)
            w_lo = g0 * J
            w_hi = min((g0 + gn) * J, nw)
            if w_lo < nw:
                nc.sync.dma_start(
                    out=out_ext[:, w_lo:w_hi], in_=maxt[:, w_lo:w_hi]
                )
            g0 += gn
    nc.finalize()
    return nc


def kernel(embedded_char, conv_w, conv_b):
    from concourse.bass_utils import run_bass_kernel_spmd

    x = np.asarray(embedded_char, np.float32)
    b_val = float(np.asarray(conv_b, np.float32).reshape(-1)[0])
    B, S, Wl, El = x.shape
    assert (Wl, El) == (W, E)
    bs = B // NCORES
    nw = bs * S
    ng = (nw + J - 1) // J
    a16 = build_conv_matrix(conv_w)

    nc = build_nc(nw)
    in_maps = [
        {
            "z": pack_input(x[i * bs : (i + 1) * bs].reshape(nw, Wl, El), ng),
            "a": a16,
        }
        for i in range(NCORES)
    ]
    res = run_bass_kernel_spmd(nc, in_maps, core_ids=list(range(NCORES)))
    full = np.concatenate(
        [r["out"].T.reshape(bs, S, El) for r in res.results], axis=0
    )
    if b_val != 0.0:
        full = full + b_val
    return np.ascontiguousarray(full.astype(np.float32))
